# revision 61
# baseline (speedup 1.0000x reference)
"""Bass/Tile kernel for causal self-attention, head-sharded across cores.

Per-core layout (core c owns heads 2c, 2c+1):
  inputs (per core):
    xT    [C, B*T]        bf16   x transposed (feature-major), same on all cores
    wqkv  [128, KC, F]    bf16   W_qkv column-slice, [p, kchunk, f]; f = [q_h0|q_h1|k_h0|k_h1|v_h0|v_h1] * 64
    wproj [128, C]        bf16   W_proj row-slice (rows = this core's 128 head dims)
    bqkv  [128, FC]       f32    b_qkv slice, partition-major per f-chunk
    pbias [128, B, T/128] f32    key-padding bias (0 or -1e30), partition-major per key chunk
    masks [128, NDIAG, TB] bf16  0/1 causal masks for diagonal tiles (offset = idx*128)
  output:
    outT  [128, C/128, B*T] bf16  partial projection output (pre-bias);
                                  host reassembles rows as c = fc*128 + p.

Schedule: QKV emitted per 512-token block (3 psum chains of 8 matmuls),
attention blocks woven between QKV blocks one-for-one so the ACT engine
(exp) stays fed while the PE stays continuously busy (p-state). Diagonal
attention tiles are trimmed to the causally-live query columns.
"""

import concourse.bass as bass
import concourse.mybir as mybir
import concourse.tile as tile
from concourse import bacc

F32 = mybir.dt.float32
BF16 = mybir.dt.bfloat16
AF = mybir.ActivationFunctionType


def build_nc(B=4, T=2048, C=1024, HPC=2, D=64, TB=512, num_devices=8,
             scale=None, pad_bias=True):
    if scale is None:
        scale = D ** -0.5
    NT = B * T                 # total tokens
    NB = NT // TB              # 512-token blocks (global)
    BPB = T // TB              # blocks per batch
    CPB = TB // 128            # 128-chunks per block (4)
    NCH = T // 128             # key chunks per batch
    KC = C // 128              # contraction chunks for qkv matmul
    F = HPC * 3 * D            # per-core qkv features (384)
    FC = F // 128              # f-chunks (3)
    FCP = KC                   # proj output f-chunks (8)
    assert HPC == 2 and HPC * D == 128 and F % 128 == 0 and TB % 128 == 0

    nc = bacc.Bacc("TRN2", target_bir_lowering=False, debug=False,
                   num_devices=num_devices)

    xT = nc.dram_tensor("xT", [C, NT], BF16, kind="ExternalInput")
    wqkv = nc.dram_tensor("wqkv", [128, KC, F], BF16, kind="ExternalInput")
    wproj = nc.dram_tensor("wproj", [128, C], BF16, kind="ExternalInput")
    bqkv = nc.dram_tensor("bqkv", [128, FC], F32, kind="ExternalInput")
    pbias = nc.dram_tensor("pbias", [128, B, NCH], F32, kind="ExternalInput")
    masks = nc.dram_tensor("masks", [128, 128], BF16, kind="ExternalInput")
    outT = nc.dram_tensor("outT", [128, FCP, NT], BF16, kind="ExternalOutput")

    with tile.TileContext(nc) as tc:
        with (
            tc.tile_pool(name="const", bufs=1) as const,
            tc.tile_pool(name="persist", bufs=1) as persist,
            tc.tile_pool(name="x0p", bufs=1) as x0p,
            tc.tile_pool(name="xp", bufs=16) as xp,
            tc.tile_pool(name="pp", bufs=8) as pp,
            tc.tile_pool(name="rp", bufs=2) as rp,
            tc.tile_pool(name="ap_", bufs=2) as ap_,
            tc.tile_pool(name="op", bufs=8) as op,
            tc.tile_pool(name="psmm", bufs=4, space="PSUM") as psmm,
            tc.tile_pool(name="pss", bufs=2, space="PSUM") as pss,
        ):
            # ---- constants (split DMAs; weights first so QKV starts fast) --
            w_sb = const.tile([128, KC, F], BF16, tag="w", name="w_sb")
            for kc in range(KC):
                eng = nc.sync if kc % 2 == 0 else nc.gpsimd
                eng.dma_start(out=w_sb[:, kc, :], in_=wqkv[:, kc, :])
            # causal triangle (p <= j), replicated for both heads; the mask
            # within the 128-col window at any diagonal offset is identical.
            # Issued from the (idle) ACT engine so it lands early.
            mk_sb = const.tile([128, HPC, 128], BF16, tag="mk", name="mk_sb")
            for h in range(HPC):
                nc.scalar.dma_start(out=mk_sb[:, h, :], in_=masks[:])
            bq_sb = const.tile([128, FC], F32, tag="bq", name="bq_sb")
            pb_sb = const.tile([128, B, NCH], F32, tag="pb", name="pb_sb")
            wp_sb = const.tile([128, C], BF16, tag="wp", name="wp_sb")
            ones_sb = const.tile([128, 64], BF16, tag="ones", name="ones_sb")
            nc.vector.memset(ones_sb[:], 1.0)
            # PE p-state warmup: chained dummy matmuls keep the PE busy while
            # the first x tiles stream in, so real work runs at full clock
            warm = psmm.tile([64, 64], F32, tag="psO", bufs=1, name="warm")
            for _ in range(170):
                nc.tensor.matmul(warm[:], lhsT=ones_sb[:, 0:64],
                                 rhs=ones_sb[:, 0:64], start=True, stop=True)

            def const_late():
                # issued after the early x loads so these bulk transfers
                # don't delay the first QKV matmuls on shared DMA queues
                nc.gpsimd.dma_start(out=wp_sb[:, 0:C // 2],
                                    in_=wproj[:, 0:C // 2])
                nc.sync.dma_start(out=wp_sb[:, C // 2:C],
                                  in_=wproj[:, C // 2:C])

            # ---- persistent per-block tiles ----
            qT = [persist.tile([128, TB], BF16, tag=f"qT{i}", name=f"qT{i}")
                  for i in range(NB)]
            kT = [persist.tile([128, TB], BF16, tag=f"kT{i}", name=f"kT{i}")
                  for i in range(NB)]
            # token-major V (2 heads stacked in free dim), one per 512-token
            # block: [128 t, CPB chunk, 128 f] filled by one batched transpose
            V = [persist.tile([128, CPB, 128], BF16, tag=f"V{i}", name=f"V{i}")
                 for i in range(NB)]

            # ---- x loads ----
            # group 0: per-(kc, block) tiles so block 0's matmuls start after
            # ~1MB of DMA; later groups: [128, 4*TB] tiles in two halves.
            GRP = 4
            ngroups = NB // GRP
            x0_tiles = {}   # (kc, tl) -> tile

            def load_group0():
                # blocks 0-1: per-(kc, block) tiles for the earliest matmuls;
                # blocks 2-3: coarser [128, 2*TB] per-kc tiles (fewer issues)
                for tl in range(2):
                    for kc in range(KC):
                        xt = x0p.tile([128, TB], BF16, tag=f"x0_{kc}_{tl}",
                                      name="x0")
                        eng = nc.sync if kc % 2 == 0 else nc.gpsimd
                        eng.dma_start(
                            out=xt[:],
                            in_=xT[kc * 128:(kc + 1) * 128,
                                   tl * TB:(tl + 1) * TB])
                        x0_tiles[(kc, tl)] = xt
                    if tl == 0:
                        # small constants ride after block 0's tiles
                        nc.sync.dma_start(out=bq_sb[:], in_=bqkv[:])
                        nc.sync.dma_start(out=pb_sb[:], in_=pbias[:])
                for kc in range(KC):
                    xt = x0p.tile([128, 2, TB], BF16, tag=f"x0b_{kc}",
                                  name="x0b")
                    eng = nc.scalar if kc % 2 == 0 else nc.gpsimd
                    eng.dma_start(
                        out=xt[:],
                        in_=xT[kc * 128:(kc + 1) * 128, 2 * TB:4 * TB])
                    for tl in (2, 3):
                        x0_tiles[(kc, tl)] = xt[:, tl - 2, :]

            x_tiles = {}    # g -> [tile per kc]

            def load_group(g):
                W = GRP * TB
                tiles = []
                for kc in range(KC):
                    xt = xp.tile([128, W], BF16, tag="xt", name="xt")
                    nc.gpsimd.dma_start(
                        out=xt[:, 0:W // 2],
                        in_=xT[kc * 128:(kc + 1) * 128,
                               g * W:g * W + W // 2])
                    nc.sync.dma_start(
                        out=xt[:, W // 2:W],
                        in_=xT[kc * 128:(kc + 1) * 128,
                               g * W + W // 2:(g + 1) * W])
                    tiles.append(xt)
                x_tiles[g] = tiles

            # ---- QKV projection, one (block, fc) chain at a time ----
            def qkv_chain(tb, fc):
                g, tl = tb // GRP, tb % GRP
                if tb % GRP == 0 and fc == 0 and g + 1 in (2, 3):
                    load_group(g + 1)
                ps = psmm.tile([128, TB], F32, tag="ps", bufs=2, name="ps")
                for kc in range(KC):
                    if g == 0:
                        rhs = x0_tiles[(kc, tl)][:]
                    else:
                        rhs = x_tiles[g][kc][:, tl * TB:(tl + 1) * TB]
                    nc.tensor.matmul(
                        ps[:], lhsT=w_sb[:, kc, fc * 128:(fc + 1) * 128],
                        rhs=rhs, start=(kc == 0), stop=(kc == KC - 1))
                if fc == 0:
                    dest = qT[tb]
                elif fc == 1:
                    dest = kT[tb]
                else:
                    dest = persist.tile([128, TB], BF16,
                                        tag=f"vs{tb % 2}", name="vs")
                # bias-add + cast on DVE (ACT is reserved for exp)
                nc.vector.tensor_scalar_add(
                    out=dest[:], in0=ps[:], scalar1=bq_sb[:, fc:fc + 1])
                if fc == 2:
                    nc.sync.dma_start_transpose(out=V[tb][:], in_=dest[:])

            # qkv chain cursor: chains are woven between attention chunk
            # pairs so the PE always has independent work while exp runs
            qkv_cursor = [0]
            NCHAINS = NB * FC

            def emit_qkv(n):
                while n > 0 and qkv_cursor[0] < NCHAINS:
                    c = qkv_cursor[0]
                    qkv_chain(c // FC, c % FC)
                    qkv_cursor[0] += 1
                    n -= 1

            # ---- attention + software-pipelined projection epilogue ----
            # Filler units (previous block's epilogue, then qkv chains) are
            # woven between each pair's S^T and PV so the PE has independent
            # work while ACT computes exp.
            from collections import deque
            filler = deque()
            points_done = [0]
            # chains spread over batches 0-2's chunk pairs plus a bit of
            # batch 3 (the per-block pre-check enforces availability order)
            PACE_DEN = sum(2 * (qb + 1) for qb in range(BPB)) * (B - 1) - 4
            BOOT = 6

            # chains for the last two blocks are held back as fill for the
            # final batch's attention (which otherwise has no PE filler)
            HOLD = NCHAINS - 2 * FC
            last_batch = [False]

            def pace(budget=4, prefer_chain=False):
                # pop ~budget matmuls' worth of filler so the PE has
                # independent work covering the exp latency of this pair
                points_done[0] += 1
                cap = NCHAINS if last_batch[0] else HOLD
                if prefer_chain and qkv_cursor[0] < cap:
                    # block start: proj units wait on the just-queued
                    # normalize; a qkv chain is ready immediately
                    emit_qkv(1)
                    budget -= KC
                while budget > 0:
                    if filler:
                        u, w, _ = filler.popleft()
                        u()
                        budget -= w if w else 1
                        continue
                    want = (BOOT + ((NCHAINS - BOOT) * points_done[0]
                                    + PACE_DEN - 1) // PACE_DEN)
                    if qkv_cursor[0] < min(want, cap):
                        emit_qkv(1)
                        budget -= KC
                    else:
                        break

            def epilogue_units(b, qb, psO, psD):
                gb = b * BPB + qb
                cell = {}

                def u_norm():
                    rt = rp.tile([128, TB], F32, tag="rt", name="rt")
                    nc.vector.reciprocal_approx_fast(out=rt[:], in_=psD[:])
                    at = ap_.tile([128, TB], BF16, tag="at", name="at")
                    nc.vector.tensor_mul(at[:], psO[:], rt[:])
                    cell["at"] = at

                def u_proj(fp):
                    def run():
                        at = cell["at"]
                        ot = op.tile([128, 2, TB], BF16, tag="ot", name="ot")
                        fine = gb == NB - 1   # last block: spread stores
                        for j in range(2):
                            fc = fp * 2 + j
                            ps = psmm.tile([128, TB], F32, tag="ps", bufs=2,
                                           name="ps")
                            nc.tensor.matmul(
                                ps[:],
                                lhsT=wp_sb[:, fc * 128:(fc + 1) * 128],
                                rhs=at[:], start=True, stop=True)
                            if fine and j == 1:
                                # last block: ACT is idle after its final
                                # exp; split the copy drain across engines
                                nc.scalar.activation(out=ot[:, j, :],
                                                     in_=ps[:], func=AF.Copy)
                            else:
                                nc.vector.tensor_copy(ot[:, j, :], ps[:])
                            if fine:
                                eng = nc.sync if j % 2 == 0 else nc.gpsimd
                                eng.dma_start(
                                    out=outT[:, fc, gb * TB:(gb + 1) * TB],
                                    in_=ot[:, j, :])
                        if not fine:
                            eng = nc.sync if fp % 2 == 0 else nc.gpsimd
                            eng.dma_start(
                                out=outT[:, fp * 2:fp * 2 + 2,
                                         gb * TB:(gb + 1) * TB],
                                in_=ot[:])
                    return run

                return [(u_norm, 0, "norm")] + [(u_proj(fp), 2, "proj")
                                                for fp in range(FCP // 2)]

            def attn_block(b, qb):
                gb = b * BPB + qb
                nchunks = (qb + 1) * CPB
                last_batch[0] = b == B - 1
                # qkv for all blocks of this batch up to qb must be emitted
                emit_qkv(FC * (b * BPB + qb + 1) - qkv_cursor[0])
                # run the previous block's normalize first so its psO/psD
                # slots free up before this block's PV/den need them
                while filler:
                    u, w, kind = filler.popleft()
                    u()
                    if kind == "norm":
                        break
                psO = psmm.tile([128, TB], F32, tag="psO", bufs=1, name="psO")
                psD = psmm.tile([128, TB], F32, tag="psD", bufs=1, name="psD")
                for c0 in range(0, nchunks, 2):
                    pts = []
                    offs = []
                    for ci in (c0, c0 + 1):
                        cb = ci // CPB      # kT block within batch
                        cl = ci % CPB       # 128-chunk within that block
                        diag = ci >= qb * CPB
                        off = (ci - qb * CPB) * 128 if diag else 0
                        offs.append(off)
                        ktile = kT[b * BPB + cb]
                        # S^T for both heads into one 2-bank PSUM tile
                        psS = pss.tile([128, HPC, TB], F32, tag="pss",
                                       name="psS")
                        for h in range(HPC):
                            nc.tensor.matmul(
                                psS[:, h, off:TB],
                                lhsT=ktile[h * 64:(h + 1) * 64,
                                           cl * 128:(cl + 1) * 128],
                                rhs=qT[gb][h * 64:(h + 1) * 64, off:TB],
                                start=True, stop=True)
                        pt = pp.tile([128, HPC, TB], BF16, tag="pt", name="pt")
                        if pad_bias:
                            nc.scalar.activation(
                                out=pt[:, :, off:TB], in_=psS[:, :, off:TB],
                                func=AF.Exp, bias=pb_sb[:, b, ci:ci + 1],
                                scale=scale)
                        else:
                            nc.scalar.activation(
                                out=pt[:, :, off:TB], in_=psS[:, :, off:TB],
                                func=AF.Exp, scale=scale)
                        pts.append(pt)
                    # fills first: their DVE ops (bias-adds, copies) have
                    # fast-ready deps and must precede the masks (which wait
                    # on exp) to avoid DVE head-of-line blocking
                    pace(prefer_chain=(c0 == 0))
                    for j, ci in enumerate((c0, c0 + 1)):
                        if ci >= qb * CPB:
                            # only cols [off, off+128) are partially masked
                            # (the same lower-triangle at any offset);
                            # beyond that the causal mask is all-ones
                            off = offs[j]
                            nc.vector.tensor_mul(
                                pts[j][:, :, off:off + 128],
                                pts[j][:, :, off:off + 128], mk_sb[:])
                    for j, ci in enumerate((c0, c0 + 1)):
                        vtile = V[b * BPB + ci // CPB]
                        off = offs[j]
                        for h in range(HPC):
                            nc.tensor.matmul(
                                psO[h * 64:(h + 1) * 64, off:TB],
                                lhsT=vtile[:, ci % CPB, h * 64:(h + 1) * 64],
                                rhs=pts[j][:, h, off:TB],
                                start=(ci == 0), stop=(ci == nchunks - 1),
                                tile_position=(0, h * 64))
                    for j, ci in enumerate((c0, c0 + 1)):
                        off = offs[j]
                        for h in range(HPC):
                            nc.tensor.matmul(
                                psD[h * 64:(h + 1) * 64, off:TB],
                                lhsT=ones_sb[:],
                                rhs=pts[j][:, h, off:TB],
                                start=(ci == 0), stop=(ci == nchunks - 1),
                                tile_position=(0, h * 64))
                # flush any leftover units of the previous block's epilogue,
                # then queue this block's epilogue for weaving into the next
                while filler:
                    filler.popleft()[0]()
                filler.extend(epilogue_units(b, qb, psO, psD))
                return None

            # ---- emission ----
            seq = [(b, qb) for b in range(B) for qb in range(BPB)]
            load_group0()
            load_group(1)
            emit_qkv(BOOT)       # bootstrap: blocks 0,1
            const_late()
            for b, qb in seq:
                attn_block(b, qb)
            while filler:
                filler.popleft()[0]()
            emit_qkv(NCHAINS)    # any stragglers (shouldn't happen)

    nc.compile()
    return nc


def prep_core_inputs(x, key_padding_mask, W_qkv, b_qkv, W_proj,
                     n_cores=8, TB=512):
    """Host-side sharding: build the per-core input maps."""
    import numpy as np
    import ml_dtypes

    B, T, C = x.shape
    D = 64
    H = C // D
    HPC = H // n_cores
    BT = B * T
    CPB = TB // 128

    xT = np.ascontiguousarray(
        x.reshape(BT, C).T).astype(ml_dtypes.bfloat16)          # [C, BT]

    pb = np.where(key_padding_mask, np.float32(-1e30),
                  np.float32(0.0)).astype(np.float32)           # [B, T]
    pb = np.ascontiguousarray(pb.reshape(B, T // 128, 128).transpose(2, 0, 1))

    p = np.arange(128)[:, None]
    j = np.arange(128)[None, :]
    mk = (p <= j).astype(ml_dtypes.bfloat16)                    # [128, 128]

    KC = C // 128
    in_maps = []
    for c in range(n_cores):
        hs = [HPC * c + i for i in range(HPC)]
        cols = np.concatenate([
            np.concatenate([which * H * D + h * D + np.arange(D) for h in hs])
            for which in range(3)])                             # [F]
        Wc = W_qkv[:, cols]                                     # [C, F]
        F = Wc.shape[1]
        wq = np.ascontiguousarray(
            Wc.reshape(KC, 128, F).transpose(1, 0, 2)).astype(ml_dtypes.bfloat16)
        bq = np.ascontiguousarray(
            b_qkv[cols].reshape(F // 128, 128).T).astype(np.float32)
        rows = np.concatenate([h * D + np.arange(D) for h in hs])
        wp = np.ascontiguousarray(W_proj[rows, :]).astype(ml_dtypes.bfloat16)
        in_maps.append({
            "xT": xT, "wqkv": wq.reshape(128, KC, F), "wproj": wp,
            "bqkv": bq, "pbias": pb, "masks": mk,
        })
    return in_maps


def combine_outputs(results, B, T, C, b_proj):
    import numpy as np
    acc = None
    for r in results:
        # outT is [128, C/128, B*T] bf16: row c = fc*128 + p
        o = np.asarray(r["outT"], dtype=np.float32)
        acc = o if acc is None else acc + o
    out = acc.transpose(1, 0, 2).reshape(C, B * T)
    out = out.T.reshape(B, T, C) + b_proj.astype(np.float32)
    return out.astype(np.float32)


# ---------------------------------------------------------------------------
# Self-contained entry point for the grading harness.
# kernel(**inputs) takes the FULL unsharded inputs and returns the FULL output.
# Sharding: tensor-parallel over heads (2 heads per core, 8 cores); each core
# computes its QKV column-slice, attention for its heads, and a partial output
# projection; partials are summed on the host.
# ---------------------------------------------------------------------------
import numpy as np

_NC_CACHE = {}


def _get_nc():
    if "nc" not in _NC_CACHE:
        _NC_CACHE["nc"] = build_nc(B=4, T=2048, C=1024, num_devices=8)
    return _NC_CACHE["nc"]


def kernel(x, key_padding_mask, W_qkv, b_qkv, W_proj, b_proj):
    from concourse.bass_utils import run_bass_kernel_spmd

    x = np.asarray(x, dtype=np.float32)
    key_padding_mask = np.asarray(key_padding_mask).astype(bool)
    W_qkv = np.asarray(W_qkv, dtype=np.float32)
    b_qkv = np.asarray(b_qkv, dtype=np.float32)
    W_proj = np.asarray(W_proj, dtype=np.float32)
    b_proj = np.asarray(b_proj, dtype=np.float32)

    B, T, C = x.shape
    nc = _get_nc()
    in_maps = prep_core_inputs(x, key_padding_mask, W_qkv, b_qkv, W_proj,
                               n_cores=8)
    res = run_bass_kernel_spmd(nc, in_maps, list(range(8)))
    return combine_outputs(res.results, B, T, C, b_proj)


# revision 63
# speedup vs baseline: 1.0148x; 1.0148x over previous
"""Bass/Tile kernel for causal self-attention, head-sharded across cores.

Per-core layout (core c owns heads 2c, 2c+1):
  inputs (per core):
    xT    [C, B*T]        bf16   x transposed (feature-major), same on all cores
    wqkv  [128, KC, F]    bf16   W_qkv column-slice, [p, kchunk, f]; f = [q_h0|q_h1|k_h0|k_h1|v_h0|v_h1] * 64
    wproj [128, C]        bf16   W_proj row-slice (rows = this core's 128 head dims)
    bqkv  [128, FC]       f32    b_qkv slice, partition-major per f-chunk
    pbias [128, B, T/128] f32    key-padding bias (0 or -1e30), partition-major per key chunk
    masks [128, NDIAG, TB] bf16  0/1 causal masks for diagonal tiles (offset = idx*128)
  output:
    outT  [128, C/128, B*T] bf16  partial projection output (pre-bias);
                                  host reassembles rows as c = fc*128 + p.

Schedule: QKV emitted per 512-token block (3 psum chains of 8 matmuls),
attention blocks woven between QKV blocks one-for-one so the ACT engine
(exp) stays fed while the PE stays continuously busy (p-state). Diagonal
attention tiles are trimmed to the causally-live query columns.
"""

import concourse.bass as bass
import concourse.mybir as mybir
import concourse.tile as tile
from concourse import bacc

F32 = mybir.dt.float32
BF16 = mybir.dt.bfloat16
AF = mybir.ActivationFunctionType


def build_nc(B=4, T=2048, C=1024, HPC=2, D=64, TB=512, num_devices=8,
             scale=None, pad_bias=True):
    if scale is None:
        scale = D ** -0.5
    NT = B * T                 # total tokens
    NB = NT // TB              # 512-token blocks (global)
    BPB = T // TB              # blocks per batch
    CPB = TB // 128            # 128-chunks per block (4)
    NCH = T // 128             # key chunks per batch
    KC = C // 128              # contraction chunks for qkv matmul
    F = HPC * 3 * D            # per-core qkv features (384)
    FC = F // 128              # f-chunks (3)
    FCP = KC                   # proj output f-chunks (8)
    assert HPC == 2 and HPC * D == 128 and F % 128 == 0 and TB % 128 == 0

    nc = bacc.Bacc("TRN2", target_bir_lowering=False, debug=False,
                   num_devices=num_devices)

    xT = nc.dram_tensor("xT", [C, NT], BF16, kind="ExternalInput")
    wqkv = nc.dram_tensor("wqkv", [128, KC, F], BF16, kind="ExternalInput")
    wproj = nc.dram_tensor("wproj", [128, C], BF16, kind="ExternalInput")
    bqkv = nc.dram_tensor("bqkv", [128, FC], F32, kind="ExternalInput")
    pbias = nc.dram_tensor("pbias", [128, B, NCH], F32, kind="ExternalInput")
    masks = nc.dram_tensor("masks", [128, 128], BF16, kind="ExternalInput")
    outT = nc.dram_tensor("outT", [128, FCP, NT], BF16, kind="ExternalOutput")

    with tile.TileContext(nc) as tc:
        with (
            tc.tile_pool(name="const", bufs=1) as const,
            tc.tile_pool(name="persist", bufs=1) as persist,
            tc.tile_pool(name="x0p", bufs=1) as x0p,
            tc.tile_pool(name="xp", bufs=16) as xp,
            tc.tile_pool(name="pp", bufs=8) as pp,
            tc.tile_pool(name="rp", bufs=2) as rp,
            tc.tile_pool(name="ap_", bufs=2) as ap_,
            tc.tile_pool(name="op", bufs=8) as op,
            tc.tile_pool(name="psmm", bufs=4, space="PSUM") as psmm,
            tc.tile_pool(name="pss", bufs=2, space="PSUM") as pss,
        ):
            # ---- constants (split DMAs; weights first so QKV starts fast) --
            w_sb = const.tile([128, KC, F], BF16, tag="w", name="w_sb")
            for kc in range(KC):
                eng = nc.sync if kc % 2 == 0 else nc.gpsimd
                eng.dma_start(out=w_sb[:, kc, :], in_=wqkv[:, kc, :])
            # causal triangle (p <= j), replicated for both heads; the mask
            # within the 128-col window at any diagonal offset is identical.
            # Issued from the (idle) ACT engine so it lands early.
            mk_sb = const.tile([128, HPC, 128], BF16, tag="mk", name="mk_sb")
            for h in range(HPC):
                nc.scalar.dma_start(out=mk_sb[:, h, :], in_=masks[:])
            bq_sb = const.tile([128, FC], F32, tag="bq", name="bq_sb")
            pb_sb = const.tile([128, B, NCH], F32, tag="pb", name="pb_sb")
            wp_sb = const.tile([128, C], BF16, tag="wp", name="wp_sb")
            ones_sb = const.tile([128, 64], BF16, tag="ones", name="ones_sb")
            nc.vector.memset(ones_sb[:], 1.0)
            # PE p-state warmup: chained dummy matmuls keep the PE busy while
            # the first x tiles stream in, so real work runs at full clock
            warm = psmm.tile([64, 64], F32, tag="psO", bufs=1, name="warm")
            for _ in range(110):
                nc.tensor.matmul(warm[:], lhsT=ones_sb[:, 0:64],
                                 rhs=ones_sb[:, 0:64], start=True, stop=True)

            def const_late():
                # issued after the early x loads so these bulk transfers
                # don't delay the first QKV matmuls on shared DMA queues
                nc.gpsimd.dma_start(out=wp_sb[:, 0:C // 2],
                                    in_=wproj[:, 0:C // 2])
                nc.sync.dma_start(out=wp_sb[:, C // 2:C],
                                  in_=wproj[:, C // 2:C])

            # ---- persistent per-block tiles ----
            qT = [persist.tile([128, TB], BF16, tag=f"qT{i}", name=f"qT{i}")
                  for i in range(NB)]
            kT = [persist.tile([128, TB], BF16, tag=f"kT{i}", name=f"kT{i}")
                  for i in range(NB)]
            # token-major V (2 heads stacked in free dim), one per 512-token
            # block: [128 t, CPB chunk, 128 f] filled by one batched transpose
            V = [persist.tile([128, CPB, 128], BF16, tag=f"V{i}", name=f"V{i}")
                 for i in range(NB)]

            # ---- x loads ----
            # group 0: per-(kc, block) tiles so block 0's matmuls start after
            # ~1MB of DMA; later groups: [128, 4*TB] tiles in two halves.
            GRP = 4
            ngroups = NB // GRP
            x0_tiles = {}   # (kc, tl) -> tile

            def load_group0():
                # blocks 0-1: per-(kc, block) tiles for the earliest matmuls;
                # blocks 2-3: coarser [128, 2*TB] per-kc tiles (fewer issues)
                for tl in range(2):
                    for kc in range(KC):
                        xt = x0p.tile([128, TB], BF16, tag=f"x0_{kc}_{tl}",
                                      name="x0")
                        eng = nc.sync if kc % 2 == 0 else nc.gpsimd
                        eng.dma_start(
                            out=xt[:],
                            in_=xT[kc * 128:(kc + 1) * 128,
                                   tl * TB:(tl + 1) * TB])
                        x0_tiles[(kc, tl)] = xt
                    if tl == 0:
                        # small constants ride after block 0's tiles
                        nc.sync.dma_start(out=bq_sb[:], in_=bqkv[:])
                        nc.sync.dma_start(out=pb_sb[:], in_=pbias[:])
                for kc in range(KC):
                    xt = x0p.tile([128, 2, TB], BF16, tag=f"x0b_{kc}",
                                  name="x0b")
                    eng = nc.scalar if kc % 2 == 0 else nc.gpsimd
                    eng.dma_start(
                        out=xt[:],
                        in_=xT[kc * 128:(kc + 1) * 128, 2 * TB:4 * TB])
                    for tl in (2, 3):
                        x0_tiles[(kc, tl)] = xt[:, tl - 2, :]

            x_tiles = {}    # g -> [tile per kc]

            def load_group(g):
                W = GRP * TB
                tiles = []
                for kc in range(KC):
                    xt = xp.tile([128, W], BF16, tag="xt", name="xt")
                    nc.gpsimd.dma_start(
                        out=xt[:, 0:W // 2],
                        in_=xT[kc * 128:(kc + 1) * 128,
                               g * W:g * W + W // 2])
                    nc.sync.dma_start(
                        out=xt[:, W // 2:W],
                        in_=xT[kc * 128:(kc + 1) * 128,
                               g * W + W // 2:(g + 1) * W])
                    tiles.append(xt)
                x_tiles[g] = tiles

            # ---- QKV projection, one (block, fc) chain at a time ----
            def qkv_chain(tb, fc):
                g, tl = tb // GRP, tb % GRP
                if tb % GRP == 0 and fc == 0 and g + 1 in (2, 3):
                    load_group(g + 1)
                ps = psmm.tile([128, TB], F32, tag="ps", bufs=2, name="ps")
                for kc in range(KC):
                    if g == 0:
                        rhs = x0_tiles[(kc, tl)][:]
                    else:
                        rhs = x_tiles[g][kc][:, tl * TB:(tl + 1) * TB]
                    nc.tensor.matmul(
                        ps[:], lhsT=w_sb[:, kc, fc * 128:(fc + 1) * 128],
                        rhs=rhs, start=(kc == 0), stop=(kc == KC - 1))
                if fc == 0:
                    dest = qT[tb]
                elif fc == 1:
                    dest = kT[tb]
                else:
                    dest = persist.tile([128, TB], BF16,
                                        tag=f"vs{tb % 2}", name="vs")
                # bias-add + cast on DVE (ACT is reserved for exp)
                nc.vector.tensor_scalar_add(
                    out=dest[:], in0=ps[:], scalar1=bq_sb[:, fc:fc + 1])
                if fc == 2:
                    nc.sync.dma_start_transpose(out=V[tb][:], in_=dest[:])

            # qkv chain cursor: chains are woven between attention chunk
            # pairs so the PE always has independent work while exp runs
            qkv_cursor = [0]
            NCHAINS = NB * FC

            def emit_qkv(n):
                while n > 0 and qkv_cursor[0] < NCHAINS:
                    c = qkv_cursor[0]
                    qkv_chain(c // FC, c % FC)
                    qkv_cursor[0] += 1
                    n -= 1

            # ---- attention + software-pipelined projection epilogue ----
            # Filler units (previous block's epilogue, then qkv chains) are
            # woven between each pair's S^T and PV so the PE has independent
            # work while ACT computes exp.
            from collections import deque
            filler = deque()
            points_done = [0]
            # chains spread over batches 0-2's chunk pairs plus a bit of
            # batch 3 (the per-block pre-check enforces availability order)
            PACE_DEN = sum(2 * (qb + 1) for qb in range(BPB)) * (B - 1) - 4
            BOOT = 6

            # chains for the last two blocks are held back as fill for the
            # final batch's attention (which otherwise has no PE filler)
            HOLD = NCHAINS - 2 * FC
            last_batch = [False]

            def pace(budget=4, prefer_chain=False):
                # pop ~budget matmuls' worth of filler so the PE has
                # independent work covering the exp latency of this pair
                points_done[0] += 1
                cap = NCHAINS if last_batch[0] else HOLD
                if prefer_chain and qkv_cursor[0] < cap:
                    # block start: proj units wait on the just-queued
                    # normalize; a qkv chain is ready immediately
                    emit_qkv(1)
                    budget -= KC
                while budget > 0:
                    if filler:
                        u, w, _ = filler.popleft()
                        u()
                        budget -= w if w else 1
                        continue
                    want = (BOOT + ((NCHAINS - BOOT) * points_done[0]
                                    + PACE_DEN - 1) // PACE_DEN)
                    if qkv_cursor[0] < min(want, cap):
                        emit_qkv(1)
                        budget -= KC
                    else:
                        break

            def epilogue_units(b, qb, psO, psD):
                gb = b * BPB + qb
                cell = {}

                def u_norm():
                    rt = rp.tile([128, TB], F32, tag="rt", name="rt")
                    nc.vector.reciprocal_approx_fast(out=rt[:], in_=psD[:])
                    at = ap_.tile([128, TB], BF16, tag="at", name="at")
                    nc.vector.tensor_mul(at[:], psO[:], rt[:])
                    cell["at"] = at

                def u_proj(fp):
                    def run():
                        at = cell["at"]
                        ot = op.tile([128, 2, TB], BF16, tag="ot", name="ot")
                        fine = gb == NB - 1   # last block: spread stores
                        for j in range(2):
                            fc = fp * 2 + j
                            ps = psmm.tile([128, TB], F32, tag="ps", bufs=2,
                                           name="ps")
                            nc.tensor.matmul(
                                ps[:],
                                lhsT=wp_sb[:, fc * 128:(fc + 1) * 128],
                                rhs=at[:], start=True, stop=True)
                            nc.vector.tensor_copy(ot[:, j, :], ps[:])
                            if fine:
                                eng = nc.sync if j % 2 == 0 else nc.gpsimd
                                eng.dma_start(
                                    out=outT[:, fc, gb * TB:(gb + 1) * TB],
                                    in_=ot[:, j, :])
                        if not fine:
                            eng = nc.sync if fp % 2 == 0 else nc.gpsimd
                            eng.dma_start(
                                out=outT[:, fp * 2:fp * 2 + 2,
                                         gb * TB:(gb + 1) * TB],
                                in_=ot[:])
                    return run

                return [(u_norm, 0, "norm")] + [(u_proj(fp), 2, "proj")
                                                for fp in range(FCP // 2)]

            def attn_block(b, qb):
                gb = b * BPB + qb
                nchunks = (qb + 1) * CPB
                last_batch[0] = b == B - 1
                # qkv for all blocks of this batch up to qb must be emitted
                emit_qkv(FC * (b * BPB + qb + 1) - qkv_cursor[0])
                # run the previous block's normalize first so its psO/psD
                # slots free up before this block's PV/den need them
                while filler:
                    u, w, kind = filler.popleft()
                    u()
                    if kind == "norm":
                        break
                psO = psmm.tile([128, TB], F32, tag="psO", bufs=1, name="psO")
                psD = psmm.tile([128, TB], F32, tag="psD", bufs=1, name="psD")
                for c0 in range(0, nchunks, 2):
                    pts = []
                    offs = []
                    for ci in (c0, c0 + 1):
                        cb = ci // CPB      # kT block within batch
                        cl = ci % CPB       # 128-chunk within that block
                        diag = ci >= qb * CPB
                        off = (ci - qb * CPB) * 128 if diag else 0
                        offs.append(off)
                        ktile = kT[b * BPB + cb]
                        # S^T for both heads into one 2-bank PSUM tile
                        psS = pss.tile([128, HPC, TB], F32, tag="pss",
                                       name="psS")
                        for h in range(HPC):
                            nc.tensor.matmul(
                                psS[:, h, off:TB],
                                lhsT=ktile[h * 64:(h + 1) * 64,
                                           cl * 128:(cl + 1) * 128],
                                rhs=qT[gb][h * 64:(h + 1) * 64, off:TB],
                                start=True, stop=True)
                        pt = pp.tile([128, HPC, TB], BF16, tag="pt", name="pt")
                        if pad_bias:
                            nc.scalar.activation(
                                out=pt[:, :, off:TB], in_=psS[:, :, off:TB],
                                func=AF.Exp, bias=pb_sb[:, b, ci:ci + 1],
                                scale=scale)
                        else:
                            nc.scalar.activation(
                                out=pt[:, :, off:TB], in_=psS[:, :, off:TB],
                                func=AF.Exp, scale=scale)
                        pts.append(pt)
                    # fills first: their DVE ops (bias-adds, copies) have
                    # fast-ready deps and must precede the masks (which wait
                    # on exp) to avoid DVE head-of-line blocking
                    pace(prefer_chain=(c0 == 0))
                    for j, ci in enumerate((c0, c0 + 1)):
                        if ci >= qb * CPB:
                            # only cols [off, off+128) are partially masked
                            # (the same lower-triangle at any offset);
                            # beyond that the causal mask is all-ones
                            off = offs[j]
                            nc.vector.tensor_mul(
                                pts[j][:, :, off:off + 128],
                                pts[j][:, :, off:off + 128], mk_sb[:])
                    for j, ci in enumerate((c0, c0 + 1)):
                        vtile = V[b * BPB + ci // CPB]
                        off = offs[j]
                        for h in range(HPC):
                            nc.tensor.matmul(
                                psO[h * 64:(h + 1) * 64, off:TB],
                                lhsT=vtile[:, ci % CPB, h * 64:(h + 1) * 64],
                                rhs=pts[j][:, h, off:TB],
                                start=(ci == 0), stop=(ci == nchunks - 1),
                                tile_position=(0, h * 64))
                    for j, ci in enumerate((c0, c0 + 1)):
                        off = offs[j]
                        for h in range(HPC):
                            nc.tensor.matmul(
                                psD[h * 64:(h + 1) * 64, off:TB],
                                lhsT=ones_sb[:],
                                rhs=pts[j][:, h, off:TB],
                                start=(ci == 0), stop=(ci == nchunks - 1),
                                tile_position=(0, h * 64))
                # flush any leftover units of the previous block's epilogue,
                # then queue this block's epilogue for weaving into the next
                while filler:
                    filler.popleft()[0]()
                filler.extend(epilogue_units(b, qb, psO, psD))
                return None

            # ---- emission ----
            seq = [(b, qb) for b in range(B) for qb in range(BPB)]
            load_group0()
            load_group(1)
            emit_qkv(BOOT)       # bootstrap: blocks 0,1
            const_late()
            for b, qb in seq:
                attn_block(b, qb)
            while filler:
                filler.popleft()[0]()
            emit_qkv(NCHAINS)    # any stragglers (shouldn't happen)

    nc.compile()
    return nc


def prep_core_inputs(x, key_padding_mask, W_qkv, b_qkv, W_proj,
                     n_cores=8, TB=512):
    """Host-side sharding: build the per-core input maps."""
    import numpy as np
    import ml_dtypes

    B, T, C = x.shape
    D = 64
    H = C // D
    HPC = H // n_cores
    BT = B * T
    CPB = TB // 128

    xT = np.ascontiguousarray(
        x.reshape(BT, C).T).astype(ml_dtypes.bfloat16)          # [C, BT]

    pb = np.where(key_padding_mask, np.float32(-1e30),
                  np.float32(0.0)).astype(np.float32)           # [B, T]
    pb = np.ascontiguousarray(pb.reshape(B, T // 128, 128).transpose(2, 0, 1))

    p = np.arange(128)[:, None]
    j = np.arange(128)[None, :]
    mk = (p <= j).astype(ml_dtypes.bfloat16)                    # [128, 128]

    KC = C // 128
    in_maps = []
    for c in range(n_cores):
        hs = [HPC * c + i for i in range(HPC)]
        cols = np.concatenate([
            np.concatenate([which * H * D + h * D + np.arange(D) for h in hs])
            for which in range(3)])                             # [F]
        Wc = W_qkv[:, cols]                                     # [C, F]
        F = Wc.shape[1]
        wq = np.ascontiguousarray(
            Wc.reshape(KC, 128, F).transpose(1, 0, 2)).astype(ml_dtypes.bfloat16)
        bq = np.ascontiguousarray(
            b_qkv[cols].reshape(F // 128, 128).T).astype(np.float32)
        rows = np.concatenate([h * D + np.arange(D) for h in hs])
        wp = np.ascontiguousarray(W_proj[rows, :]).astype(ml_dtypes.bfloat16)
        in_maps.append({
            "xT": xT, "wqkv": wq.reshape(128, KC, F), "wproj": wp,
            "bqkv": bq, "pbias": pb, "masks": mk,
        })
    return in_maps


def combine_outputs(results, B, T, C, b_proj):
    import numpy as np
    acc = None
    for r in results:
        # outT is [128, C/128, B*T] bf16: row c = fc*128 + p
        o = np.asarray(r["outT"], dtype=np.float32)
        acc = o if acc is None else acc + o
    out = acc.transpose(1, 0, 2).reshape(C, B * T)
    out = out.T.reshape(B, T, C) + b_proj.astype(np.float32)
    return out.astype(np.float32)


# ---------------------------------------------------------------------------
# Self-contained entry point for the grading harness.
# kernel(**inputs) takes the FULL unsharded inputs and returns the FULL output.
# Sharding: tensor-parallel over heads (2 heads per core, 8 cores); each core
# computes its QKV column-slice, attention for its heads, and a partial output
# projection; partials are summed on the host.
# ---------------------------------------------------------------------------
import numpy as np

_NC_CACHE = {}


def _get_nc():
    if "nc" not in _NC_CACHE:
        _NC_CACHE["nc"] = build_nc(B=4, T=2048, C=1024, num_devices=8)
    return _NC_CACHE["nc"]


def kernel(x, key_padding_mask, W_qkv, b_qkv, W_proj, b_proj):
    from concourse.bass_utils import run_bass_kernel_spmd

    x = np.asarray(x, dtype=np.float32)
    key_padding_mask = np.asarray(key_padding_mask).astype(bool)
    W_qkv = np.asarray(W_qkv, dtype=np.float32)
    b_qkv = np.asarray(b_qkv, dtype=np.float32)
    W_proj = np.asarray(W_proj, dtype=np.float32)
    b_proj = np.asarray(b_proj, dtype=np.float32)

    B, T, C = x.shape
    nc = _get_nc()
    in_maps = prep_core_inputs(x, key_padding_mask, W_qkv, b_qkv, W_proj,
                               n_cores=8)
    res = run_bass_kernel_spmd(nc, in_maps, list(range(8)))
    return combine_outputs(res.results, B, T, C, b_proj)


# revision 65
# speedup vs baseline: 1.0356x; 1.0205x over previous
"""Bass/Tile kernel for causal self-attention, head-sharded across cores.

Per-core layout (core c owns heads 2c, 2c+1):
  inputs (per core):
    xT    [C, B*T]        bf16   x transposed (feature-major), same on all cores
    wqkv  [128, KC, F]    bf16   W_qkv column-slice, [p, kchunk, f]; f = [q_h0|q_h1|k_h0|k_h1|v_h0|v_h1] * 64
    wproj [128, C]        bf16   W_proj row-slice (rows = this core's 128 head dims)
    bqkv  [128, FC]       f32    b_qkv slice, partition-major per f-chunk
    pbias [128, B, T/128] f32    key-padding bias (0 or -1e30), partition-major per key chunk
    masks [128, NDIAG, TB] bf16  0/1 causal masks for diagonal tiles (offset = idx*128)
  output:
    outT  [128, C/128, B*T] bf16  partial projection output (pre-bias);
                                  host reassembles rows as c = fc*128 + p.

Schedule: QKV emitted per 512-token block (3 psum chains of 8 matmuls),
attention blocks woven between QKV blocks one-for-one so the ACT engine
(exp) stays fed while the PE stays continuously busy (p-state). Diagonal
attention tiles are trimmed to the causally-live query columns.
"""

import concourse.bass as bass
import concourse.mybir as mybir
import concourse.tile as tile
from concourse import bacc

F32 = mybir.dt.float32
BF16 = mybir.dt.bfloat16
AF = mybir.ActivationFunctionType


def build_nc(B=4, T=2048, C=1024, HPC=2, D=64, TB=512, num_devices=8,
             scale=None, pad_bias=True):
    if scale is None:
        scale = D ** -0.5
    NT = B * T                 # total tokens
    NB = NT // TB              # 512-token blocks (global)
    BPB = T // TB              # blocks per batch
    CPB = TB // 128            # 128-chunks per block (4)
    NCH = T // 128             # key chunks per batch
    KC = C // 128              # contraction chunks for qkv matmul
    F = HPC * 3 * D            # per-core qkv features (384)
    FC = F // 128              # f-chunks (3)
    FCP = KC                   # proj output f-chunks (8)
    assert HPC == 2 and HPC * D == 128 and F % 128 == 0 and TB % 128 == 0

    nc = bacc.Bacc("TRN2", target_bir_lowering=False, debug=False,
                   num_devices=num_devices)

    xT = nc.dram_tensor("xT", [C, NT], BF16, kind="ExternalInput")
    wqkv = nc.dram_tensor("wqkv", [128, KC, F], BF16, kind="ExternalInput")
    wproj = nc.dram_tensor("wproj", [128, C], BF16, kind="ExternalInput")
    bqkv = nc.dram_tensor("bqkv", [128, FC], F32, kind="ExternalInput")
    pbias = nc.dram_tensor("pbias", [128, B, NCH], F32, kind="ExternalInput")
    masks = nc.dram_tensor("masks", [128, 128], BF16, kind="ExternalInput")
    outT = nc.dram_tensor("outT", [128, FCP, NT], BF16, kind="ExternalOutput")

    with tile.TileContext(nc) as tc:
        with (
            tc.tile_pool(name="const", bufs=1) as const,
            tc.tile_pool(name="persist", bufs=1) as persist,
            tc.tile_pool(name="x0p", bufs=1) as x0p,
            tc.tile_pool(name="xp", bufs=16) as xp,
            tc.tile_pool(name="pp", bufs=8) as pp,
            tc.tile_pool(name="rp", bufs=2) as rp,
            tc.tile_pool(name="ap_", bufs=2) as ap_,
            tc.tile_pool(name="op", bufs=8) as op,
            tc.tile_pool(name="psmm", bufs=4, space="PSUM") as psmm,
            tc.tile_pool(name="pss", bufs=2, space="PSUM") as pss,
        ):
            # ---- constants (split DMAs, interleaved with block-0 x below) --
            w_sb = const.tile([128, KC, F], BF16, tag="w", name="w_sb")
            # causal triangle (p <= j), replicated for both heads; the mask
            # within the 128-col window at any diagonal offset is identical.
            # Issued from the (idle) ACT engine so it lands early.
            mk_sb = const.tile([128, HPC, 128], BF16, tag="mk", name="mk_sb")
            for h in range(HPC):
                nc.scalar.dma_start(out=mk_sb[:, h, :], in_=masks[:])
            bq_sb = const.tile([128, FC], F32, tag="bq", name="bq_sb")
            pb_sb = const.tile([128, B, NCH], F32, tag="pb", name="pb_sb")
            wp_sb = const.tile([128, C], BF16, tag="wp", name="wp_sb")
            ones_sb = const.tile([128, 64], BF16, tag="ones", name="ones_sb")
            nc.vector.memset(ones_sb[:], 1.0)
            # PE p-state warmup: chained dummy matmuls keep the PE busy while
            # the first x tiles stream in, so real work runs at full clock
            warm = psmm.tile([64, 64], F32, tag="psO", bufs=1, name="warm")
            for _ in range(110):
                nc.tensor.matmul(warm[:], lhsT=ones_sb[:, 0:64],
                                 rhs=ones_sb[:, 0:64], start=True, stop=True)

            def const_late():
                # issued after the early x loads so these bulk transfers
                # don't delay the first QKV matmuls on shared DMA queues
                nc.gpsimd.dma_start(out=wp_sb[:, 0:C // 2],
                                    in_=wproj[:, 0:C // 2])
                nc.sync.dma_start(out=wp_sb[:, C // 2:C],
                                  in_=wproj[:, C // 2:C])

            # ---- persistent per-block tiles ----
            qT = [persist.tile([128, TB], BF16, tag=f"qT{i}", name=f"qT{i}")
                  for i in range(NB)]
            kT = [persist.tile([128, TB], BF16, tag=f"kT{i}", name=f"kT{i}")
                  for i in range(NB)]
            # token-major V (2 heads stacked in free dim), one per 512-token
            # block: [128 t, CPB chunk, 128 f] filled by one batched transpose
            V = [persist.tile([128, CPB, 128], BF16, tag=f"V{i}", name=f"V{i}")
                 for i in range(NB)]

            # ---- x loads ----
            # group 0: per-(kc, block) tiles so block 0's matmuls start after
            # ~1MB of DMA; later groups: [128, 4*TB] tiles in two halves.
            GRP = 4
            ngroups = NB // GRP
            x0_tiles = {}   # (kc, tl) -> tile

            def load_group0():
                # blocks 0-1: per-(kc, block) tiles for the earliest matmuls;
                # blocks 2-3: coarser [128, 2*TB] per-kc tiles (fewer issues)
                for tl in range(2):
                    for kc in range(KC):
                        xt = x0p.tile([128, TB], BF16, tag=f"x0_{kc}_{tl}",
                                      name="x0")
                        eng = nc.sync if kc % 2 == 0 else nc.gpsimd
                        eng.dma_start(
                            out=xt[:],
                            in_=xT[kc * 128:(kc + 1) * 128,
                                   tl * TB:(tl + 1) * TB])
                        x0_tiles[(kc, tl)] = xt
                        if tl == 0:
                            # weight slice rides right behind its x tile
                            eng2 = nc.gpsimd if kc % 2 == 0 else nc.sync
                            eng2.dma_start(out=w_sb[:, kc, :],
                                           in_=wqkv[:, kc, :])
                    if tl == 0:
                        # small constants ride after block 0's tiles
                        nc.sync.dma_start(out=bq_sb[:], in_=bqkv[:])
                        nc.sync.dma_start(out=pb_sb[:], in_=pbias[:])
                for kc in range(KC):
                    xt = x0p.tile([128, 2, TB], BF16, tag=f"x0b_{kc}",
                                  name="x0b")
                    eng = nc.scalar if kc % 2 == 0 else nc.gpsimd
                    eng.dma_start(
                        out=xt[:],
                        in_=xT[kc * 128:(kc + 1) * 128, 2 * TB:4 * TB])
                    for tl in (2, 3):
                        x0_tiles[(kc, tl)] = xt[:, tl - 2, :]

            x_tiles = {}    # g -> [tile per kc]

            def load_group(g):
                W = GRP * TB
                tiles = []
                for kc in range(KC):
                    xt = xp.tile([128, W], BF16, tag="xt", name="xt")
                    nc.gpsimd.dma_start(
                        out=xt[:, 0:W // 2],
                        in_=xT[kc * 128:(kc + 1) * 128,
                               g * W:g * W + W // 2])
                    nc.sync.dma_start(
                        out=xt[:, W // 2:W],
                        in_=xT[kc * 128:(kc + 1) * 128,
                               g * W + W // 2:(g + 1) * W])
                    tiles.append(xt)
                x_tiles[g] = tiles

            # ---- QKV projection, one (block, fc) chain at a time ----
            def qkv_chain(tb, fc):
                g, tl = tb // GRP, tb % GRP
                if tb % GRP == 0 and fc == 0 and g + 1 in (2, 3):
                    load_group(g + 1)
                ps = psmm.tile([128, TB], F32, tag="ps", bufs=2, name="ps")
                for kc in range(KC):
                    if g == 0:
                        rhs = x0_tiles[(kc, tl)][:]
                    else:
                        rhs = x_tiles[g][kc][:, tl * TB:(tl + 1) * TB]
                    nc.tensor.matmul(
                        ps[:], lhsT=w_sb[:, kc, fc * 128:(fc + 1) * 128],
                        rhs=rhs, start=(kc == 0), stop=(kc == KC - 1))
                if fc == 0:
                    dest = qT[tb]
                elif fc == 1:
                    dest = kT[tb]
                else:
                    dest = persist.tile([128, TB], BF16,
                                        tag=f"vs{tb % 2}", name="vs")
                # bias-add + cast on DVE (ACT is reserved for exp)
                nc.vector.tensor_scalar_add(
                    out=dest[:], in0=ps[:], scalar1=bq_sb[:, fc:fc + 1])
                if fc == 2:
                    nc.sync.dma_start_transpose(out=V[tb][:], in_=dest[:])

            # qkv chain cursor: chains are woven between attention chunk
            # pairs so the PE always has independent work while exp runs
            qkv_cursor = [0]
            NCHAINS = NB * FC

            def emit_qkv(n):
                while n > 0 and qkv_cursor[0] < NCHAINS:
                    c = qkv_cursor[0]
                    qkv_chain(c // FC, c % FC)
                    qkv_cursor[0] += 1
                    n -= 1

            # ---- attention + software-pipelined projection epilogue ----
            # Filler units (previous block's epilogue, then qkv chains) are
            # woven between each pair's S^T and PV so the PE has independent
            # work while ACT computes exp.
            from collections import deque
            filler = deque()
            points_done = [0]
            # chains spread over batches 0-2's chunk pairs plus a bit of
            # batch 3 (the per-block pre-check enforces availability order)
            PACE_DEN = sum(2 * (qb + 1) for qb in range(BPB)) * (B - 1) - 4
            BOOT = 6

            # chains for the last two blocks are held back as fill for the
            # final batch's attention (which otherwise has no PE filler)
            HOLD = NCHAINS - 2 * FC
            last_batch = [False]

            def pace(budget=4, prefer_chain=False):
                # pop ~budget matmuls' worth of filler so the PE has
                # independent work covering the exp latency of this pair
                points_done[0] += 1
                cap = NCHAINS if last_batch[0] else HOLD
                if prefer_chain and qkv_cursor[0] < cap:
                    # block start: proj units wait on the just-queued
                    # normalize; a qkv chain is ready immediately
                    emit_qkv(1)
                    budget -= KC
                while budget > 0:
                    if filler:
                        u, w, _ = filler.popleft()
                        u()
                        budget -= w if w else 1
                        continue
                    want = (BOOT + ((NCHAINS - BOOT) * points_done[0]
                                    + PACE_DEN - 1) // PACE_DEN)
                    if qkv_cursor[0] < min(want, cap):
                        emit_qkv(1)
                        budget -= KC
                    else:
                        break

            def epilogue_units(b, qb, psO, psD):
                gb = b * BPB + qb
                cell = {}

                def u_norm():
                    rt = rp.tile([128, TB], F32, tag="rt", name="rt")
                    nc.vector.reciprocal_approx_fast(out=rt[:], in_=psD[:])
                    at = ap_.tile([128, TB], BF16, tag="at", name="at")
                    nc.vector.tensor_mul(at[:], psO[:], rt[:])
                    cell["at"] = at

                def u_proj(fp):
                    def run():
                        at = cell["at"]
                        ot = op.tile([128, 2, TB], BF16, tag="ot", name="ot")
                        fine = gb == NB - 1   # last block: spread stores
                        for j in range(2):
                            fc = fp * 2 + j
                            ps = psmm.tile([128, TB], F32, tag="ps", bufs=2,
                                           name="ps")
                            nc.tensor.matmul(
                                ps[:],
                                lhsT=wp_sb[:, fc * 128:(fc + 1) * 128],
                                rhs=at[:], start=True, stop=True)
                            nc.vector.tensor_copy(ot[:, j, :], ps[:])
                            if fine:
                                eng = nc.sync if j % 2 == 0 else nc.gpsimd
                                eng.dma_start(
                                    out=outT[:, fc, gb * TB:(gb + 1) * TB],
                                    in_=ot[:, j, :])
                        if not fine:
                            eng = nc.sync if fp % 2 == 0 else nc.gpsimd
                            eng.dma_start(
                                out=outT[:, fp * 2:fp * 2 + 2,
                                         gb * TB:(gb + 1) * TB],
                                in_=ot[:])
                    return run

                return [(u_norm, 0, "norm")] + [(u_proj(fp), 2, "proj")
                                                for fp in range(FCP // 2)]

            def attn_block(b, qb):
                gb = b * BPB + qb
                nchunks = (qb + 1) * CPB
                last_batch[0] = b == B - 1
                # qkv for all blocks of this batch up to qb must be emitted
                emit_qkv(FC * (b * BPB + qb + 1) - qkv_cursor[0])
                # run the previous block's normalize first so its psO/psD
                # slots free up before this block's PV/den need them
                while filler:
                    u, w, kind = filler.popleft()
                    u()
                    if kind == "norm":
                        break
                psO = psmm.tile([128, TB], F32, tag="psO", bufs=1, name="psO")
                psD = psmm.tile([128, TB], F32, tag="psD", bufs=1, name="psD")
                for c0 in range(0, nchunks, 2):
                    pts = []
                    offs = []
                    for ci in (c0, c0 + 1):
                        cb = ci // CPB      # kT block within batch
                        cl = ci % CPB       # 128-chunk within that block
                        diag = ci >= qb * CPB
                        off = (ci - qb * CPB) * 128 if diag else 0
                        offs.append(off)
                        ktile = kT[b * BPB + cb]
                        # S^T for both heads into one 2-bank PSUM tile
                        psS = pss.tile([128, HPC, TB], F32, tag="pss",
                                       name="psS")
                        for h in range(HPC):
                            nc.tensor.matmul(
                                psS[:, h, off:TB],
                                lhsT=ktile[h * 64:(h + 1) * 64,
                                           cl * 128:(cl + 1) * 128],
                                rhs=qT[gb][h * 64:(h + 1) * 64, off:TB],
                                start=True, stop=True)
                        pt = pp.tile([128, HPC, TB], BF16, tag="pt", name="pt")
                        if pad_bias:
                            nc.scalar.activation(
                                out=pt[:, :, off:TB], in_=psS[:, :, off:TB],
                                func=AF.Exp, bias=pb_sb[:, b, ci:ci + 1],
                                scale=scale)
                        else:
                            nc.scalar.activation(
                                out=pt[:, :, off:TB], in_=psS[:, :, off:TB],
                                func=AF.Exp, scale=scale)
                        pts.append(pt)
                    # fills first: their DVE ops (bias-adds, copies) have
                    # fast-ready deps and must precede the masks (which wait
                    # on exp) to avoid DVE head-of-line blocking
                    pace(prefer_chain=(c0 == 0))
                    for j, ci in enumerate((c0, c0 + 1)):
                        if ci >= qb * CPB:
                            # only cols [off, off+128) are partially masked
                            # (the same lower-triangle at any offset);
                            # beyond that the causal mask is all-ones
                            off = offs[j]
                            nc.vector.tensor_mul(
                                pts[j][:, :, off:off + 128],
                                pts[j][:, :, off:off + 128], mk_sb[:])
                    for j, ci in enumerate((c0, c0 + 1)):
                        vtile = V[b * BPB + ci // CPB]
                        off = offs[j]
                        for h in range(HPC):
                            nc.tensor.matmul(
                                psO[h * 64:(h + 1) * 64, off:TB],
                                lhsT=vtile[:, ci % CPB, h * 64:(h + 1) * 64],
                                rhs=pts[j][:, h, off:TB],
                                start=(ci == 0), stop=(ci == nchunks - 1),
                                tile_position=(0, h * 64))
                    for j, ci in enumerate((c0, c0 + 1)):
                        off = offs[j]
                        for h in range(HPC):
                            nc.tensor.matmul(
                                psD[h * 64:(h + 1) * 64, off:TB],
                                lhsT=ones_sb[:],
                                rhs=pts[j][:, h, off:TB],
                                start=(ci == 0), stop=(ci == nchunks - 1),
                                tile_position=(0, h * 64))
                # flush any leftover units of the previous block's epilogue,
                # then queue this block's epilogue for weaving into the next
                while filler:
                    filler.popleft()[0]()
                filler.extend(epilogue_units(b, qb, psO, psD))
                return None

            # ---- emission ----
            seq = [(b, qb) for b in range(B) for qb in range(BPB)]
            load_group0()
            load_group(1)
            emit_qkv(BOOT)       # bootstrap: blocks 0,1
            const_late()
            for b, qb in seq:
                attn_block(b, qb)
            while filler:
                filler.popleft()[0]()
            emit_qkv(NCHAINS)    # any stragglers (shouldn't happen)

    nc.compile()
    return nc


def prep_core_inputs(x, key_padding_mask, W_qkv, b_qkv, W_proj,
                     n_cores=8, TB=512):
    """Host-side sharding: build the per-core input maps."""
    import numpy as np
    import ml_dtypes

    B, T, C = x.shape
    D = 64
    H = C // D
    HPC = H // n_cores
    BT = B * T
    CPB = TB // 128

    xT = np.ascontiguousarray(
        x.reshape(BT, C).T).astype(ml_dtypes.bfloat16)          # [C, BT]

    pb = np.where(key_padding_mask, np.float32(-1e30),
                  np.float32(0.0)).astype(np.float32)           # [B, T]
    pb = np.ascontiguousarray(pb.reshape(B, T // 128, 128).transpose(2, 0, 1))

    p = np.arange(128)[:, None]
    j = np.arange(128)[None, :]
    mk = (p <= j).astype(ml_dtypes.bfloat16)                    # [128, 128]

    KC = C // 128
    in_maps = []
    for c in range(n_cores):
        hs = [HPC * c + i for i in range(HPC)]
        cols = np.concatenate([
            np.concatenate([which * H * D + h * D + np.arange(D) for h in hs])
            for which in range(3)])                             # [F]
        Wc = W_qkv[:, cols]                                     # [C, F]
        F = Wc.shape[1]
        wq = np.ascontiguousarray(
            Wc.reshape(KC, 128, F).transpose(1, 0, 2)).astype(ml_dtypes.bfloat16)
        bq = np.ascontiguousarray(
            b_qkv[cols].reshape(F // 128, 128).T).astype(np.float32)
        rows = np.concatenate([h * D + np.arange(D) for h in hs])
        wp = np.ascontiguousarray(W_proj[rows, :]).astype(ml_dtypes.bfloat16)
        in_maps.append({
            "xT": xT, "wqkv": wq.reshape(128, KC, F), "wproj": wp,
            "bqkv": bq, "pbias": pb, "masks": mk,
        })
    return in_maps


def combine_outputs(results, B, T, C, b_proj):
    import numpy as np
    acc = None
    for r in results:
        # outT is [128, C/128, B*T] bf16: row c = fc*128 + p
        o = np.asarray(r["outT"], dtype=np.float32)
        acc = o if acc is None else acc + o
    out = acc.transpose(1, 0, 2).reshape(C, B * T)
    out = out.T.reshape(B, T, C) + b_proj.astype(np.float32)
    return out.astype(np.float32)


# ---------------------------------------------------------------------------
# Self-contained entry point for the grading harness.
# kernel(**inputs) takes the FULL unsharded inputs and returns the FULL output.
# Sharding: tensor-parallel over heads (2 heads per core, 8 cores); each core
# computes its QKV column-slice, attention for its heads, and a partial output
# projection; partials are summed on the host.
# ---------------------------------------------------------------------------
import numpy as np

_NC_CACHE = {}


def _get_nc():
    if "nc" not in _NC_CACHE:
        _NC_CACHE["nc"] = build_nc(B=4, T=2048, C=1024, num_devices=8)
    return _NC_CACHE["nc"]


def kernel(x, key_padding_mask, W_qkv, b_qkv, W_proj, b_proj):
    from concourse.bass_utils import run_bass_kernel_spmd

    x = np.asarray(x, dtype=np.float32)
    key_padding_mask = np.asarray(key_padding_mask).astype(bool)
    W_qkv = np.asarray(W_qkv, dtype=np.float32)
    b_qkv = np.asarray(b_qkv, dtype=np.float32)
    W_proj = np.asarray(W_proj, dtype=np.float32)
    b_proj = np.asarray(b_proj, dtype=np.float32)

    B, T, C = x.shape
    nc = _get_nc()
    in_maps = prep_core_inputs(x, key_padding_mask, W_qkv, b_qkv, W_proj,
                               n_cores=8)
    res = run_bass_kernel_spmd(nc, in_maps, list(range(8)))
    return combine_outputs(res.results, B, T, C, b_proj)
